# revision 21
# baseline (speedup 1.0000x reference)
"""DSC layer (moe_routing) on 8 TRN2 NeuronCores, data-parallel over tokens.

Math (per token n):
  r0[nb]   = sum_d x[n,d]*g[d]*rW[nb,d]            (bf16 matmul)
  r_raw    = rs[n]*r0 - rs[n]*mu[n]*sg[nb] + c[nb] (LN folded into scalars)
  alpha    = softplus(clip(r_raw, +-10))
  top-8 of alpha via HW max8 + match_replace -> masked alpha (Zscat)
  q[n]     = tanh(S)/(S+eps), S = sum of top-8
  h_full   = x @ U_norm.T ; G = Zscat*q*h_full
  dyn      = G @ (V_norm * gamma)   (accumulated into the same PSUM as static)
  static   = gelu(x@W1.T) @ W2.T
All matmuls bf16 (fp32 accum). Stats (mean/var) computed in f32 via bn_stats.
All transposed layouts are prepared host-side (free); only math runs on device.
"""
import sys, os
sys.path.insert(0, "/opt/trn_rl_repo")
from contextlib import ExitStack
import numpy as np
import concourse.bass as bass
import concourse.mybir as mybir
from concourse import bacc
from concourse.tile import TileContext
from concourse.bass_utils import run_bass_kernel_spmd

F32 = mybir.dt.float32
BF16 = mybir.dt.bfloat16
AF = mybir.ActivationFunctionType
OP = mybir.AluOpType
AX = mybir.AxisListType

D, NB, H = 1024, 512, 4096
NCORE = 8
T = 1024          # tokens per core
P = 128
TI = T // P       # 8 token tiles
DK = D // P       # 8 contraction tiles over D
HJ = H // P       # 32 tiles over ffn hidden
NBJ = NB // P     # 4 tiles over basis dim
TAU = 10.0
EPS = 1e-6
GELU = (AF.Identity if os.environ.get("KERNEL_NO_GELU") else AF.Gelu)
KPHASE = os.environ.get("KPHASE", "full")
KSUB = int(os.environ.get("KSUB", "99"))


def _build():
    nc = bacc.Bacc("TRN2", target_bir_lowering=False, debug=False, num_devices=NCORE)
    xt_e = nc.declare_dram_parameter("xt", [D, T], F32, isOutput=False)
    xn_e = nc.declare_dram_parameter("xn", [T, D], F32, isOutput=False)
    w1t_e = nc.declare_dram_parameter("w1t", [D, H], F32, isOutput=False)
    w2t_e = nc.declare_dram_parameter("w2t", [H, D], F32, isOutput=False)
    rwt_e = nc.declare_dram_parameter("rwt", [D, NB], F32, isOutput=False)
    ut_e = nc.declare_dram_parameter("ut", [D, NB], F32, isOutput=False)
    v_e = nc.declare_dram_parameter("v", [NB, D], F32, isOutput=False)
    gcol_e = nc.declare_dram_parameter("gcol", [P, DK], F32, isOutput=False)
    bcol_e = nc.declare_dram_parameter("bcol", [P, DK], F32, isOutput=False)
    rb_e = nc.declare_dram_parameter("rb", [1, NB], F32, isOutput=False)
    gam_e = nc.declare_dram_parameter("gam", [1, D], F32, isOutput=False)
    eye_e = nc.declare_dram_parameter("eye", [P, P], F32, isOutput=False)
    out_e = nc.declare_dram_parameter("out", [T, D], F32, isOutput=True)

    xt_v = xt_e[:].rearrange("(ko p) t -> p ko t", p=P)      # [128, DK, T]
    xn_v = xn_e[:].rearrange("(to p) d -> p to d", p=P)      # [128, TI, D]
    w1t_v = w1t_e[:].rearrange("(ko p) h -> p ko h", p=P)    # [128, DK, H]
    w2t_v = w2t_e[:].rearrange("(ho p) d -> p ho d", p=P)    # [128, HJ, D]
    rwt_v = rwt_e[:].rearrange("(ko p) n -> p ko n", p=P)    # [128, DK, NB]
    ut_v = ut_e[:].rearrange("(ko p) n -> p ko n", p=P)      # [128, DK, NB]
    v_v = v_e[:].rearrange("(no p) d -> p no d", p=P)        # [128, NBJ, D]
    out_v = out_e[:].rearrange("(to p) d -> p to d", p=P)    # [128, TI, D]

    with TileContext(nc, pool_alloc_mode="queue") as tc, ExitStack() as ctx:
        # ---- persistent pools (live through the whole kernel) ----
        const = ctx.enter_context(tc.tile_pool(name="const", bufs=1))
        persist = ctx.enter_context(tc.tile_pool(name="persist", bufs=1))
        big = ctx.enter_context(tc.tile_pool(name="big", bufs=1))

        eyef = const.tile([P, P], F32)
        nc.scalar.dma_start(eyef[:], eye_e[:])
        ident = const.tile([P, P], BF16)
        nc.vector.tensor_copy(ident[:], eyef[:])
        ones_row = const.tile([1, P], BF16)
        nc.vector.memset(ones_row[:], 1.0)
        ones_bc = const.tile([P, P], BF16)
        nc.vector.memset(ones_bc[:], 1.0)
        epsb = const.tile([P, 1], F32)
        nc.vector.memset(epsb[:], 1e-5)
        gcol = const.tile([P, DK], F32)
        bcol = const.tile([P, DK], F32)
        rb_row = const.tile([1, NB], F32)
        gam_row = const.tile([1, D], F32)
        nc.sync.dma_start(gcol[:], gcol_e[:])
        nc.sync.dma_start(bcol[:], bcol_e[:])
        nc.sync.dma_start(rb_row[:], rb_e[:])
        nc.sync.dma_start(gam_row[:], gam_e[:])

        # resident tensors
        xtb = persist.tile([P, DK, T], BF16)          # x.T bf16, 2MB
        wg = persist.tile([P, DK, NB], BF16)          # (g*rW).T bf16, 1MB
        un = persist.tile([P, DK, NB], BF16)          # U_norm.T bf16, 1MB
        vs = persist.tile([P, NBJ, D], BF16)          # V_norm*gamma bf16, 1MB
        gt = persist.tile([P, NBJ, T], BF16)          # G.T bf16, 1MB
        gelu_h = big.tile([P, HJ, T], BF16)           # gelu(x@W1.T).T bf16, 8MB
        w2c = big.tile([P, HJ, D], BF16)              # W2.T bf16, 8MB
        sg_b = persist.tile([P, NB], F32)             # sum_d g*rW, bcast over parts
        c_b = persist.tile([P, NB], F32)              # sum_d b*rW + rb, bcast
        gam_b = persist.tile([P, D], F32)             # gamma bcast over parts
        rs_t = persist.tile([P, TI], F32)             # 1/sqrt(var+1e-5) per token
        mrs_t = persist.tile([P, TI], F32)            # -mu*rs per token

        # ================= phase 0: load + prep =================
        with tc.tile_pool(name="p0", bufs=2) as p0, \
             tc.tile_pool(name="p0b", bufs=2) as p0b, \
             tc.tile_pool(name="pp0", bufs=2, space="PSUM") as pp0:
            # x.T load + cast, stats from natural layout
            xtf = p0.tile([P, DK, T], F32, tag="xtf")
            nc.sync.dma_start(xtf[:], xt_v[:])
            for dk in range(DK):
                nc.vector.tensor_copy(xtb[:, dk, :], xtf[:, dk, :])

            bnst = const.tile([P, 2, 6], F32)
            bnag = const.tile([P, 2], F32)
            lnv = const.tile([P, TI], F32)
            for ti in range(TI):
                xnt = p0b.tile([P, D], F32, tag="xnt")
                nc.gpsimd.dma_start(xnt[:], xn_v[:, ti, :])
                nc.vector.bn_stats(bnst[:, 0], xnt[:, 0:512])
                nc.vector.bn_stats(bnst[:, 1], xnt[:, 512:1024])
                nc.vector.bn_aggr(bnag[:], bnst[:])
                # rs = (var+1e-5)^-0.5 = exp(-0.5*ln(var+1e-5)); mrs = -mean*rs
                nc.scalar.activation(lnv[:, ti : ti + 1], bnag[:, 1:2], AF.Ln,
                                     bias=epsb[:])
                nc.scalar.activation(rs_t[:, ti : ti + 1], lnv[:, ti : ti + 1],
                                     AF.Exp, scale=-0.5)
                nc.vector.scalar_tensor_tensor(
                    mrs_t[:, ti : ti + 1], bnag[:, 0:1], -1.0,
                    rs_t[:, ti : ti + 1], OP.mult, OP.mult)

            # router table prep
            rwtf = p0.tile([P, DK, NB], F32, tag="rwtf")
            nc.scalar.dma_start(rwtf[:], rwt_v[:])
            for dk in range(DK):
                nc.vector.tensor_scalar(wg[:, dk, :], rwtf[:, dk, :],
                                        gcol[:, dk : dk + 1], None, OP.mult)
            # g_bc / b_bc broadcast cols -> [P, P] for ones-style matmuls (f32)
            gbc = p0.tile([P, DK, P], F32, tag="gbc")
            bbc = p0.tile([P, DK, P], F32, tag="bbc")
            for dk in range(DK):
                nc.vector.tensor_copy(gbc[:, dk, :],
                                      gcol[:, dk : dk + 1].to_broadcast([P, P]))
                nc.vector.tensor_copy(bbc[:, dk, :],
                                      bcol[:, dk : dk + 1].to_broadcast([P, P]))
            # sg_b = sum_d g*rW (partition-bcast); c_b = sum_d b*rW + rb
            sg_ps = pp0.tile([P, NB], F32, tag="ps512")
            for dk in range(DK):
                nc.tensor.matmul(sg_ps[:], gbc[:, dk, :], rwtf[:, dk, :],
                                 start=(dk == 0), stop=(dk == DK - 1))
            nc.vector.tensor_copy(sg_b[:], sg_ps[:])
            c_ps = pp0.tile([P, NB], F32, tag="ps512")
            for dk in range(DK):
                nc.tensor.matmul(c_ps[:], bbc[:, dk, :], rwtf[:, dk, :],
                                 start=(dk == 0), stop=False)
            nc.tensor.matmul(c_ps[:], ones_row[:], rb_row[:], start=False, stop=True)
            nc.vector.tensor_copy(c_b[:], c_ps[:])

            # gamma partition-broadcast [P, D]
            for half in range(2):
                gam_ps = pp0.tile([P, 512], F32, tag="ps512")
                nc.tensor.matmul(gam_ps[:], ones_row[:],
                                 gam_row[:, half * 512 : (half + 1) * 512],
                                 start=True, stop=True)
                nc.vector.tensor_copy(gam_b[:, half * 512 : (half + 1) * 512],
                                      gam_ps[:])

            # U norms: sum_d U^2 via ones-matmul on squares (f32), then scale
            utf = p0.tile([P, DK, NB], F32, tag="utf")
            nc.scalar.dma_start(utf[:], ut_v[:])
            nsq_ps = pp0.tile([P, NB], F32, tag="ps512")
            for dk in range(DK):
                usq = p0b.tile([P, NB], F32, tag="usq")
                nc.vector.tensor_tensor(usq[:], utf[:, dk, :], utf[:, dk, :], OP.mult)
                nc.tensor.matmul(nsq_ps[:], ones_bc[:], usq[:],
                                 start=(dk == 0), stop=(dk == DK - 1))
            rno = const.tile([P, NB], F32)
            nc.scalar.activation(rno[:], nsq_ps[:], AF.Ln)
            nc.scalar.activation(rno[:], rno[:], AF.Exp, scale=-0.5)
            nc.vector.tensor_scalar_min(rno[:], rno[:], 1.0 / EPS)
            for dk in range(DK):
                nc.vector.tensor_tensor(un[:, dk, :], utf[:, dk, :], rno[:], OP.mult)

            # V norms (free-axis) + gamma fold
            vf = p0.tile([P, NBJ, D], F32, tag="vf")
            nc.scalar.dma_start(vf[:], v_v[:])
            vss = const.tile([P, NBJ], F32)
            rnv = const.tile([P, NBJ], F32)
            for nbj in range(NBJ):
                vsq = p0b.tile([P, D], F32, tag="vsq")
                nc.vector.tensor_tensor_reduce(
                    vsq[:], vf[:, nbj, :], vf[:, nbj, :], 1.0, 0.0,
                    OP.mult, OP.add, accum_out=vss[:, nbj : nbj + 1])
            nc.scalar.activation(rnv[:], vss[:], AF.Ln)
            nc.scalar.activation(rnv[:], rnv[:], AF.Exp, scale=-0.5)
            nc.vector.tensor_scalar_min(rnv[:], rnv[:], 1.0 / EPS)
            for nbj in range(NBJ):
                nc.vector.scalar_tensor_tensor(
                    vs[:, nbj, :], vf[:, nbj, :], rnv[:, nbj : nbj + 1],
                    gam_b[:], OP.mult, OP.mult)

        # ================= phase A: router + h_full + topk =================
        with tc.tile_pool(name="pa", bufs=3) as pa, \
             tc.tile_pool(name="pa_sm", bufs=4) as pa_sm, \
             tc.tile_pool(name="ppa", bufs=6, space="PSUM") as ppa, \
             tc.tile_pool(name="ppt", bufs=3, space="PSUM") as ppt:
            for ti in range(TI):
                tsl = slice(ti * P, (ti + 1) * P)
                r0 = ppa.tile([P, NB], F32, tag="pA")
                hf = ppa.tile([P, NB], F32, tag="pA")
                for dk in range(DK):
                    nc.tensor.matmul(r0[:], xtb[:, dk, tsl], wg[:, dk, :],
                                     start=(dk == 0), stop=(dk == DK - 1))
                for dk in range(DK):
                    nc.tensor.matmul(hf[:], xtb[:, dk, tsl], un[:, dk, :],
                                     start=(dk == 0), stop=(dk == DK - 1))
                # LN fixup: r = rs*r0 + c - mu*rs*sg, then clip, softplus
                rf = pa.tile([P, NB], F32, tag="rf")
                nc.vector.scalar_tensor_tensor(rf[:], r0[:], rs_t[:, ti : ti + 1],
                                               c_b[:], OP.mult, OP.add)
                nc.vector.scalar_tensor_tensor(rf[:], sg_b[:], mrs_t[:, ti : ti + 1],
                                               rf[:], OP.mult, OP.add)
                nc.vector.tensor_scalar(rf[:], rf[:], TAU, -TAU, OP.min, OP.max)
                e_sb = pa.tile([P, NB], F32, tag="e_sb")
                nc.scalar.activation(e_sb[:], rf[:], AF.Exp)
                alpha = pa.tile([P, NB], F32, tag="alpha")
                nc.scalar.activation(alpha[:], e_sb[:], AF.Ln, bias=1.0)
                m8 = pa_sm.tile([P, 8], F32, tag="m8")
                nc.vector.max(out=m8[:], in_=alpha[:])
                ssum = pa_sm.tile([P, 1], F32, tag="ssum")
                nc.vector.reduce_sum(ssum[:], m8[:], axis=AX.X)
                sm = pa_sm.tile([P, 1], F32, tag="sm")
                nc.vector.tensor_scalar(sm[:], ssum[:], 2.0, 60.0, OP.mult, OP.min)
                e2s = pa_sm.tile([P, 1], F32, tag="e2s")
                nc.scalar.activation(e2s[:], sm[:], AF.Exp)
                den = pa_sm.tile([P, 1], F32, tag="den")
                nc.vector.tensor_scalar_add(den[:], ssum[:], EPS)
                nc.vector.scalar_tensor_tensor(den[:], e2s[:], 1.0, den[:],
                                               OP.add, OP.mult)
                nc.vector.reciprocal(den[:], den[:])
                q = pa_sm.tile([P, 1], F32, tag="q")
                nc.vector.scalar_tensor_tensor(q[:], e2s[:], -1.0, den[:],
                                               OP.add, OP.mult)
                repl = pa.tile([P, NB], F32, tag="repl")
                nc.vector.match_replace(out=repl[:], in_to_replace=m8[:],
                                        in_values=alpha[:], imm_value=0.0)
                zs = pa.tile([P, NB], F32, tag="zs")
                nc.vector.tensor_sub(zs[:], alpha[:], repl[:])
                g_sb = pa.tile([P, NB], BF16, tag="g_sb")
                nc.vector.scalar_tensor_tensor(g_sb[:], zs[:], q[:], hf[:],
                                               OP.mult, OP.mult)
                for nbj in range(NBJ):
                    pt = ppt.tile([P, P], BF16, tag="pt")
                    nc.tensor.transpose(pt[:], g_sb[:, nbj * P : (nbj + 1) * P],
                                        ident[:])
                    nc.vector.tensor_copy(gt[:, nbj, tsl], pt[:])

        # ================= phase B: FFN1 + gelu; stream W2 cast =================
        with tc.tile_pool(name="pb", bufs=6) as pb, \
             tc.tile_pool(name="pw2", bufs=3) as pw2, \
             tc.tile_pool(name="ppb", bufs=2, space="PSUM") as ppb:
            W2CH = 2  # hj tiles per w2 staging chunk
            n_w2_chunks = HJ // W2CH  # 16
            for hj in range(HJ):
                w1f = pb.tile([P, DK, P], F32, tag="w1f")
                nc.sync.dma_start(w1f[:], w1t_v[:, :, hj * P : (hj + 1) * P])
                w1c = pb.tile([P, DK, P], BF16, tag="w1c")
                nc.vector.tensor_copy(
                    w1c[:].rearrange("p a b -> p (a b)"),
                    w1f[:].rearrange("p a b -> p (a b)"))
                hps = ppb.tile([P, T], F32, tag="hps")
                for dk in range(DK):
                    nc.tensor.matmul(hps[:, 0:512], w1c[:, dk, :],
                                     xtb[:, dk, 0:512],
                                     start=(dk == 0), stop=(dk == DK - 1))
                    nc.tensor.matmul(hps[:, 512:1024], w1c[:, dk, :],
                                     xtb[:, dk, 512:1024],
                                     start=(dk == 0), stop=(dk == DK - 1))
                nc.scalar.activation(gelu_h[:, hj, :], hps[:], AF.Gelu)
                # interleave one W2 chunk load+cast every other hj
                if hj % 2 == 0:
                    ch = hj // 2
                    if ch < n_w2_chunks:
                        hsl = slice(ch * W2CH, (ch + 1) * W2CH)
                        w2f = pw2.tile([P, W2CH, D], F32, tag="w2f")
                        nc.sync.dma_start(w2f[:], w2t_v[:, hsl, :])
                        nc.scalar.copy(
                            w2c[:, hsl, :].rearrange("p a b -> p (a b)"),
                            w2f[:].rearrange("p a b -> p (a b)"))

        # ================= phase C: dyn + FFN2 into one PSUM =================
        with tc.tile_pool(name="pc", bufs=2) as pc, \
             tc.tile_pool(name="ppc", bufs=3, space="PSUM") as ppc:
            for ti in range(TI):
                tsl = slice(ti * P, (ti + 1) * P)
                ops = ppc.tile([P, D], F32, tag="ops")
                for half in range(2):
                    dsl = slice(half * 512, (half + 1) * 512)
                    for nbj in range(NBJ):
                        nc.tensor.matmul(ops[:, dsl], gt[:, nbj, tsl],
                                         vs[:, nbj, dsl],
                                         start=(nbj == 0), stop=False)
                    for hj in range(HJ):
                        nc.tensor.matmul(ops[:, dsl], gelu_h[:, hj, tsl],
                                         w2c[:, hj, dsl],
                                         start=False, stop=(hj == HJ - 1))
                o_sb = pc.tile([P, D], F32, tag="o_sb")
                nc.vector.tensor_copy(o_sb[:], ops[:])
                nc.sync.dma_start(out_v[:, ti, :], o_sb[:])

    nc.compile()
    return nc


_cached_nc = None
_EYE = np.eye(P, dtype=np.float32)


def kernel(x, W1, W2, ln_g, ln_b, router_W, router_b, raw_U, raw_V, gamma):
    global _cached_nc
    x = np.ascontiguousarray(np.asarray(x, np.float32)).reshape(-1, D)
    w1t = np.ascontiguousarray(np.asarray(W1, np.float32).T)
    w2t = np.ascontiguousarray(np.asarray(W2, np.float32).T)
    rwt = np.ascontiguousarray(np.asarray(router_W, np.float32).T)
    utt = np.ascontiguousarray(np.asarray(raw_U, np.float32).T)
    vv = np.ascontiguousarray(np.asarray(raw_V, np.float32))
    gcol = np.ascontiguousarray(np.asarray(ln_g, np.float32).reshape(DK, P).T)
    bcol = np.ascontiguousarray(np.asarray(ln_b, np.float32).reshape(DK, P).T)
    rb = np.ascontiguousarray(np.asarray(router_b, np.float32).reshape(1, NB))
    gam = np.ascontiguousarray(np.asarray(gamma, np.float32).reshape(1, D))

    if _cached_nc is None:
        _cached_nc = _build()
    nc = _cached_nc

    in_maps = []
    for c in range(NCORE):
        shard = x[c * T : (c + 1) * T]
        in_maps.append({
            "xt": np.ascontiguousarray(shard.T), "xn": np.ascontiguousarray(shard),
            "w1t": w1t, "w2t": w2t, "rwt": rwt, "ut": utt, "v": vv,
            "gcol": gcol, "bcol": bcol, "rb": rb, "gam": gam,
            "eye": _EYE,
        })
    res = run_bass_kernel_spmd(nc, in_maps, list(range(NCORE)))
    kernel._last_results = res
    out = np.concatenate([res.results[c]["out"] for c in range(NCORE)], axis=0)
    return out.reshape(4, 2048, D)
